# revision 11
# baseline (speedup 1.0000x reference)
"""Trainium2 Bass kernel for nn_RNNModel_16509854285990.

2-layer tanh RNN (B=64, T=2048, F=256, H0=H1=512) + linear head -> [B, 128].

Strategy: data-parallel over batch (8 cores x B_local=8). Per core:
  phase 1: xp0^T = W_ih0 @ x^T + b         (bulk matmul, bf16)
  phase 2: layer-0 recurrence  h_t = tanh(xp0_t + W_hh0 @ h_{t-1})
           state kept transposed [feat(part) x batch(free)] so the ACT
           output feeds the next step's moving operand directly (no
           transpose anywhere). xp_t injected into PSUM via a single
           identity matmul, then 16 bf16 W-matmuls accumulate, then one
           ACT tanh per step.
  phase 3: xp1^T = W_ih1 @ h0^T + b        (bulk matmul, bf16)
  phase 4: layer-1 recurrence (only final hidden kept, in fp32)
  phase 5: head  out^T = W_fc @ h_T + b_fc (fp32)

All shapes/strategy are hardcoded for this problem.
"""

import os
import numpy as np
import ml_dtypes

B, T_FULL, F, H, O = 64, 2048, 256, 512, 128
NCORES = 8
BL = B // NCORES            # batch per core
KF = F // 128               # 2 k-tiles for x features
KH = H // 128               # 4 k/m-tiles for hidden features
MB = KH * BL                # free elems per step tile (32)
CH = 64                     # time steps per DMA chunk

_bf = ml_dtypes.bfloat16


def _build_nc(T):
    import concourse.bacc as bacc
    import concourse.mybir as mybir
    from concourse.tile import TileContext

    fp32 = mybir.dt.float32
    bf16 = mybir.dt.bfloat16
    Tanh = mybir.ActivationFunctionType.Tanh
    Ident = mybir.ActivationFunctionType.Identity
    NCH = T // CH           # chunks

    nc = bacc.Bacc()

    # ---- external inputs (per-core arrays supplied via in_maps) ----
    xT = nc.declare_dram_parameter("xT", [BL, KF, 128, T], bf16, False)
    wih0 = nc.declare_dram_parameter("wih0", [KF, 128, KH, 128], bf16, False)
    whh0 = nc.declare_dram_parameter("whh0", [KH, 128, KH, 128], bf16, False)
    wih1 = nc.declare_dram_parameter("wih1", [KH, 128, KH, 128], bf16, False)
    whh1 = nc.declare_dram_parameter("whh1", [KH, 128, KH, 128], bf16, False)
    wfc = nc.declare_dram_parameter("wfc", [KH, 128, O], fp32, False)
    bias0 = nc.declare_dram_parameter("bias0", [KH, 128], fp32, False)
    bias1 = nc.declare_dram_parameter("bias1", [KH, 128], fp32, False)
    bfc = nc.declare_dram_parameter("bfc", [O, 1], fp32, False)
    ident = nc.declare_dram_parameter("ident", [128, 128], bf16, False)
    h0t0 = nc.declare_dram_parameter("h0t0", [128, MB], bf16, False)
    h0t1 = nc.declare_dram_parameter("h0t1", [128, MB], bf16, False)
    out = nc.declare_dram_parameter("out", [BL, O], fp32, True)

    # ---- internal DRAM intermediates (chunked ring layout [c,p,m,b,t]) ----
    xp0 = nc.dram_tensor("xp0", [NCH, 128, KH, BL, CH], bf16)
    xp1 = nc.dram_tensor("xp1", [NCH, 128, KH, BL, CH], bf16)
    h0s = nc.dram_tensor("h0s", [NCH, 128, KH, BL, CH], bf16)

    TC = T // 512 if T >= 512 else 1     # 512-token column chunks per b
    TCW = min(T, 512)                    # token-chunk width
    CPT = TCW // CH                      # chunks per token-chunk

    with TileContext(nc) as tc:
        # ================= phase 1 + 3: input projections =================
        def projection(src_tiles_dma, w_sb, bias_sb, dst, KIN):
            """dst[c,p,m,b,t] (bf16) = sum_k w[k,m].T @ rhs_k + bias, per b."""
            with (
                tc.tile_pool(name="proj_rhs", bufs=6) as rhsp,
                tc.tile_pool(name="proj_ps", bufs=8, space="PSUM") as psp,
                tc.tile_pool(name="proj_ev", bufs=4) as evp,
            ):
                for b in range(BL):
                    rts = []
                    for k in range(KIN):
                        rt = rhsp.tile([128, T], bf16, tag="rhs")
                        src_tiles_dma(rt, b, k)
                        rts.append(rt)
                    for tcix in range(TC):
                        for m in range(KH):
                            ps = psp.tile([128, TCW], fp32, tag="ps")
                            for k in range(KIN):
                                nc.tensor.matmul(
                                    ps,
                                    lhsT=w_sb[:, (k * KH + m) * 128:(k * KH + m + 1) * 128],
                                    rhs=rts[k][:, tcix * TCW:(tcix + 1) * TCW],
                                    start=(k == 0),
                                    stop=(k == KIN - 1),
                                )
                            ev = evp.tile([128, TCW], bf16, tag="ev")
                            if (m + tcix) % 2 == 0:
                                nc.scalar.activation(ev, ps, Ident, bias=bias_sb[:, m:m + 1])
                            else:
                                nc.vector.tensor_scalar_add(ev, ps, bias_sb[:, m:m + 1])
                            nc.sync.dma_start(
                                out=dst[tcix * CPT:(tcix + 1) * CPT, :, m, b, :]
                                .rearrange("c p t -> p c t"),
                                in_=ev.rearrange("p (c t) -> p c t", c=CPT),
                            )

        # ================= recurrence =================
        def recurrence(xp_src, whh_sb, ident_sb, h0t_sb, hout_dst, hfin_tile):
            with (
                tc.tile_pool(name="rec_ring", bufs=3) as ringp,
                tc.tile_pool(name="rec_hst", bufs=3) as hstp,
                tc.tile_pool(name="rec_ps", bufs=4, space="PSUM") as psp,
            ):
                prev_hst = None
                for c in range(NCH):
                    ring = ringp.tile([128, KH * BL * CH], bf16, tag="ring")
                    nc.sync.dma_start(
                        out=ring, in_=xp_src[c].rearrange("p m b t -> p (m b t)")
                    )
                    rv = ring.rearrange("p (m b t) -> p m b t", m=KH, b=BL)
                    hst = hstp.tile([128, KH * BL * CH], bf16, tag="hst")
                    hv = hst.rearrange("p (m b t) -> p m b t", m=KH, b=BL)
                    for ti in range(CH):
                        gstep = c * CH + ti
                        last = gstep == T - 1
                        if gstep == 0:
                            hprev = lambda k: h0t_sb[:, k * BL:(k + 1) * BL]
                        elif ti == 0:
                            pv = prev_hst.rearrange(
                                "p (m b t) -> p m b t", m=KH, b=BL)
                            hprev = lambda k, pv=pv: pv[:, k, :, CH - 1]
                        else:
                            hprev = lambda k, hv=hv, ti=ti: hv[:, k, :, ti - 1]

                        # Two PSUM banks per step so the tanh of the first
                        # half (m 0-1) overlaps the PE stream of the second
                        # half, and both ACTs hide under the next step's
                        # leading matmuls (same-bank PE-write + ACT-read is
                        # fatal, hence the split).
                        half = MB // 2
                        bankA = psp.tile([128, half], fp32, tag="bankA")
                        bankB = psp.tile([128, half], fp32, tag="bankB")
                        banks = [bankA, bankB]
                        for h_ in range(2):
                            nc.tensor.matmul(
                                banks[h_], lhsT=ident_sb,
                                rhs=rv[:, 2 * h_:2 * h_ + 2, :, ti]
                                .rearrange("p m b -> p (m b)"),
                                start=True, stop=False,
                            )
                        for m in range(KH):
                            bk = banks[m // 2]
                            off = (m % 2) * BL
                            for k in range(KH):
                                nc.tensor.matmul(
                                    bk[:, off:off + BL],
                                    lhsT=whh_sb[:, (k * KH + m) * 128:(k * KH + m + 1) * 128],
                                    rhs=hprev(k),
                                    start=False,
                                    stop=(m % 2 == 1 and k == KH - 1),
                                )
                            if m % 2 == 1:
                                h_ = m // 2
                                if last and hfin_tile is not None:
                                    nc.scalar.activation(
                                        hfin_tile[:, h_ * 2 * BL:(h_ + 1) * 2 * BL],
                                        banks[h_], Tanh)
                                    nc.vector.tensor_copy(
                                        hv[:, 2 * h_:2 * h_ + 2, :, ti]
                                        .rearrange("p m b -> p (m b)"),
                                        hfin_tile[:, h_ * 2 * BL:(h_ + 1) * 2 * BL])
                                else:
                                    nc.scalar.activation(
                                        hv[:, 2 * h_:2 * h_ + 2, :, ti]
                                        .rearrange("p m b -> p (m b)"),
                                        banks[h_], Tanh)
                    if hout_dst is not None:
                        nc.sync.dma_start(
                            out=hout_dst[c].rearrange("p m b t -> p (m b t)"),
                            in_=hst,
                        )
                    prev_hst = hst

        # ================= constants in SBUF =================
        with (
            tc.tile_pool(name="consts", bufs=1) as cst,
        ):
            # Warmup ACTs with <=1 sem wait so walrus attaches the activation
            # table-load pseudos here (the first real ACTs already carry two
            # waits; a merged table load overflows the ISA sync-wait slots).
            scratch = cst.tile([128, 1], fp32, tag="scratch")
            c0 = nc.const_aps.tensor(0.0, (128, 1))
            nc.scalar.activation(scratch, c0, Tanh)
            nc.scalar.activation(scratch, c0, Ident, bias=0.0)

            wih0_sb = cst.tile([128, KF * KH * 128], bf16, tag="wih0")
            nc.sync.dma_start(out=wih0_sb.rearrange("p (k m q) -> p k m q", k=KF, m=KH), in_=wih0.rearrange("k p m q -> p k m q"))
            whh0_sb = cst.tile([128, KH * KH * 128], bf16, tag="whh0")
            nc.sync.dma_start(out=whh0_sb.rearrange("p (k m q) -> p k m q", k=KH, m=KH), in_=whh0.rearrange("k p m q -> p k m q"))
            wih1_sb = cst.tile([128, KH * KH * 128], bf16, tag="wih1")
            nc.sync.dma_start(out=wih1_sb.rearrange("p (k m q) -> p k m q", k=KH, m=KH), in_=wih1.rearrange("k p m q -> p k m q"))
            whh1_sb = cst.tile([128, KH * KH * 128], bf16, tag="whh1")
            nc.sync.dma_start(out=whh1_sb.rearrange("p (k m q) -> p k m q", k=KH, m=KH), in_=whh1.rearrange("k p m q -> p k m q"))
            wfc_sb = cst.tile([128, KH * O], fp32, tag="wfc")
            nc.sync.dma_start(out=wfc_sb.rearrange("p (k o) -> p k o", k=KH), in_=wfc.rearrange("k p o -> p k o"))
            bias0_sb = cst.tile([128, KH], fp32, tag="bias0")
            nc.sync.dma_start(out=bias0_sb, in_=bias0.rearrange("k p -> p k"))
            bias1_sb = cst.tile([128, KH], fp32, tag="bias1")
            nc.sync.dma_start(out=bias1_sb, in_=bias1.rearrange("k p -> p k"))
            bfc_sb = cst.tile([128, 1], fp32, tag="bfc")
            nc.sync.dma_start(out=bfc_sb, in_=bfc[:, :])
            ident_sb = cst.tile([128, 128], bf16, tag="ident")
            nc.sync.dma_start(out=ident_sb, in_=ident[:, :])
            h0t0_sb = cst.tile([128, MB], bf16, tag="h0t0")
            nc.sync.dma_start(out=h0t0_sb, in_=h0t0[:, :])
            h0t1_sb = cst.tile([128, MB], bf16, tag="h0t1")
            nc.sync.dma_start(out=h0t1_sb, in_=h0t1[:, :])
            h1fin = cst.tile([128, MB], fp32, tag="h1fin")
            outT = cst.tile([128, BL], fp32, tag="outT")

            # phase 1: xp0 from x
            def x_dma(rt, b, k):
                nc.sync.dma_start(out=rt, in_=xT[b, k])
            projection(x_dma, wih0_sb, bias0_sb, xp0, KF)

            # phase 2: layer-0 recurrence
            recurrence(xp0, whh0_sb, ident_sb, h0t0_sb, h0s, None)

            # phase 3: xp1 from h0s
            def h_dma(rt, b, k):
                nc.sync.dma_start(
                    out=rt.rearrange("p (c t) -> p c t", c=T // CH),
                    in_=h0s[:, :, k, b, :].rearrange("c p t -> p c t"))
            projection(h_dma, wih1_sb, bias1_sb, xp1, KH)

            # phase 4: layer-1 recurrence
            recurrence(xp1, whh1_sb, ident_sb, h0t1_sb, None, h1fin)

            # phase 5: head
            with tc.tile_pool(name="head_ps", bufs=1, space="PSUM") as hps:
                psh = hps.tile([128, BL], fp32, tag="psh")
                for k in range(KH):
                    nc.tensor.matmul(
                        psh,
                        lhsT=wfc_sb[:, k * O:(k + 1) * O],
                        rhs=h1fin[:, k * BL:(k + 1) * BL],
                        start=(k == 0),
                        stop=(k == KH - 1),
                    )
                nc.scalar.activation(outT, psh, Ident, bias=bfc_sb[:, 0:1])
                nc.sync.dma_start(out=out.rearrange("b o -> o b"), in_=outT)

    nc.compile()
    return nc


def _prep_core_inputs(inputs, core, T):
    """Build the in_map dict for one core from full-size inputs."""
    f32 = np.float32
    xs = np.ascontiguousarray(inputs["x"][core * BL:(core + 1) * BL, :T])  # [BL,T,F]
    xT = np.ascontiguousarray(xs.transpose(0, 2, 1)).reshape(BL, KF, 128, T)

    def wT(w, kin):  # w: [H_out, F_in] -> lhsT tiles [kin,128,KH,128]
        return np.ascontiguousarray(w.T).reshape(kin, 128, KH, 128)

    def h0tile(h0):
        t = np.ascontiguousarray(h0.reshape(KH, 128).T)      # [128, KH]
        return np.repeat(t[:, :, None], BL, axis=2).reshape(128, MB)

    return {
        "xT": xT.astype(_bf),
        "wih0": wT(inputs["W_ih0"], KF).astype(_bf),
        "whh0": wT(inputs["W_hh0"], KH).astype(_bf),
        "wih1": wT(inputs["W_ih1"], KH).astype(_bf),
        "whh1": wT(inputs["W_hh1"], KH).astype(_bf),
        "wfc": np.ascontiguousarray(inputs["W_fc"].T).reshape(KH, 128, O).astype(f32),
        "bias0": (inputs["b_ih0"] + inputs["b_hh0"]).reshape(KH, 128).astype(f32),
        "bias1": (inputs["b_ih1"] + inputs["b_hh1"]).reshape(KH, 128).astype(f32),
        "bfc": inputs["b_fc"].reshape(O, 1).astype(f32),
        "ident": np.eye(128).astype(_bf),
        "h0t0": h0tile(inputs["h0_0"].astype(f32)).astype(_bf),
        "h0t1": h0tile(inputs["h0_1"].astype(f32)).astype(_bf),
    }


_NC_CACHE = {}


def _get_nc(T):
    if T not in _NC_CACHE:
        _NC_CACHE[T] = _build_nc(T)
    return _NC_CACHE[T]


def kernel(**inputs):
    T = int(os.environ.get("RNN_T", T_FULL))
    inputs = {k: np.asarray(v) for k, v in inputs.items()}
    nc = _get_nc(T)
    in_maps = [_prep_core_inputs(inputs, i, T) for i in range(NCORES)]

    if os.environ.get("RNN_SIM"):
        from concourse.bass_interp import CoreSim
        sim = CoreSim(nc, require_finite=False)
        for name, arr in in_maps[0].items():
            sim.tensor(name)[:] = arr
        sim.simulate()
        out0 = np.array(sim.tensor("out"))
        full = np.zeros((B, O), np.float32)
        full[0:BL] = out0
        return full

    from concourse.bass_utils import run_bass_kernel_spmd
    kwargs = {}
    if os.environ.get("RNN_PROFILE"):
        kwargs = dict(trace=True, tmpdir=os.environ.get("RNN_TRACE_DIR") or None)
    res = run_bass_kernel_spmd(nc, in_maps, list(range(NCORES)), **kwargs)
    if os.environ.get("RNN_PROFILE"):
        print(f"HW exec time: {res.exec_time_ns} ns")
        print(f"trace: {res.instructions_and_trace}")
    outs = [res.results[i]["out"] for i in range(NCORES)]
    return np.concatenate(outs, axis=0).astype(np.float32)
